# revision 4
# baseline (speedup 1.0000x reference)
"""Trainium2 Bass kernel for nn_GraphUnrollingDen (gnn_message_passing).

conv[i,j] = sum_l A^l[i,j] * MLP_l(P[i] - P[j]),  l = 0..2

Key algebra: MLP_l first layer is linear in (P[i]-P[j]), so with
U_l = P @ W1[l]  (tiny, host-precomputed):
    m_l[i,j] = sum_k W2[l,k] * relu(U_l[i,k] - U_l[j,k] + b1[l,k]) + b2[l]
l=0 uses A^0 = I, so it only contributes a constant c0 on the diagonal.

Device work per core (rows sharded 8 ways, 192 rows each):
  - A^2 row-block via f32r GEMM (full-rate fp32 path on the PE).
  - m_l for l=1,2: elementwise relu on ScalarE/VectorE/GPSIMD with the
    per-partition bias trick (partitions = 32 i's x 4 k's), then the
    k-reduction as 8 accumulating bf16 matmuls per 32-i block, col-tiled
    via tile_position so PSUM fills all 128 partitions.
  - combine: conv = A.rows * m1 + A^2.rows * m2 on VectorE, DMA out.
"""

import sys

if "/opt/trn_rl_repo" not in sys.path:
    sys.path.insert(0, "/opt/trn_rl_repo")

import numpy as np
import ml_dtypes

import concourse.bacc as bacc
import concourse.mybir as mybir
import concourse.tile as tile
from concourse.bass_utils import run_bass_kernel_spmd

BF16 = ml_dtypes.bfloat16

N = 1536          # nodes
KP = 32           # p (eigenvector count) == contraction dim of MLP layer 1
FTS = 32          # MLP hidden width (unused on device after U-trick)
L = 3             # adjacency powers
NC = 8            # cores
R = N // NC       # rows per core = 192
NBLK = R // 32    # 32-i blocks per core = 6
NJC = 3           # j chunks of 512
JC = 512
NT = 8            # k chunks of 4
NL = 2            # device-computed powers: l = 1, 2

_PROG_CACHE = {}


def _build_program():
    nc = bacc.Bacc("TRN2")
    dt = mybir.dt

    a_full = nc.dram_tensor("a_full", [N, N], dt.float32r, kind="ExternalInput")
    a_colT = nc.dram_tensor("a_colT", [N, R], dt.float32r, kind="ExternalInput")
    a_rows = nc.dram_tensor("a_rows", [R, N], dt.float32, kind="ExternalInput")
    uh_d = nc.dram_tensor("uh", [NL * NT, 128, N], dt.bfloat16, kind="ExternalInput")
    bias_d = nc.dram_tensor("bias", [128, NL * NT * NBLK], dt.float32, kind="ExternalInput")
    wblk_d = nc.dram_tensor("wblk", [128, NL * NT * 32], dt.bfloat16, kind="ExternalInput")
    conv_d = nc.dram_tensor("conv_out", [R, N], dt.float32, kind="ExternalOutput")

    f32, f32r, bf = dt.float32, dt.float32r, dt.bfloat16
    KC = N // 128  # 12 contraction chunks for the GEMM

    with tile.TileContext(nc) as tc:
        with tc.tile_pool(name="const", bufs=1) as const_pool, \
             tc.tile_pool(name="arows", bufs=1) as arows_pool, \
             tc.tile_pool(name="a2", bufs=1) as a2_pool:

            # ---- resident tiles ----
            uh_sb = const_pool.tile([128, NL * NT, N], bf)
            for lt in range(NL * NT):
                nc.sync.dma_start(out=uh_sb[:, lt, :], in_=uh_d[lt])
            bias_sb = const_pool.tile([128, NL * NT * NBLK], f32)
            nc.sync.dma_start(out=bias_sb[:], in_=bias_d.ap())
            wblk_sb = const_pool.tile([128, NL * NT * 32], bf)
            nc.sync.dma_start(out=wblk_sb[:], in_=wblk_d.ap())

            ar = [arows_pool.tile([128, N], f32, tag="ar0", name="ar0"),
                  arows_pool.tile([64, N], f32, tag="ar1", name="ar1")]
            nc.sync.dma_start(out=ar[0][:], in_=a_rows[0:128, :])
            nc.sync.dma_start(out=ar[1][:], in_=a_rows[128:192, :])

            a2 = [a2_pool.tile([128, N], f32, tag="a20", name="a20"),
                  a2_pool.tile([64, N], f32, tag="a21", name="a21")]

            # ---- phase 1: A^2 row-block GEMM (f32r full-rate) ----
            with tc.tile_pool(name="grhs", bufs=4) as grhs_pool, \
                 tc.tile_pool(name="glhs", bufs=4) as glhs_pool, \
                 tc.tile_pool(name="gps", bufs=1, space="PSUM") as gps_pool:
                ps = []
                for mt in range(2):
                    rows = 128 if mt == 0 else 64
                    for jc in range(NJC):
                        ps.append(gps_pool.tile([rows, JC], f32, tag=f"g{mt}{jc}", name=f"gps{mt}{jc}"))
                for kc in range(KC):
                    rhs = grhs_pool.tile([128, N], f32r)
                    nc.sync.dma_start(out=rhs[:], in_=a_full[kc * 128:(kc + 1) * 128, :])
                    lhsT = glhs_pool.tile([128, R], f32r)
                    nc.sync.dma_start(out=lhsT[:], in_=a_colT[kc * 128:(kc + 1) * 128, :])
                    for mt in range(2):
                        msl = slice(0, 128) if mt == 0 else slice(128, 192)
                        for jc in range(NJC):
                            nc.tensor.matmul(
                                ps[mt * NJC + jc][:],
                                lhsT=lhsT[:, msl],
                                rhs=rhs[:, jc * JC:(jc + 1) * JC],
                                start=(kc == 0), stop=(kc == KC - 1),
                                skip_group_check=True,
                            )
                for mt in range(2):
                    for jc in range(NJC):
                        nc.scalar.copy(a2[mt][:, jc * JC:(jc + 1) * JC], ps[mt * NJC + jc][:])

            # ---- phase 2: MLP m_l + combine ----
            # engine pattern for the relu/bias elementwise op (index % 16)
            pat = "DDADGDDADGDDADGD"
            with tc.tile_pool(name="h", bufs=10) as h_pool, \
                 tc.tile_pool(name="mps", bufs=4, space="PSUM") as mps_pool, \
                 tc.tile_pool(name="cv", bufs=4) as cv_pool:
                eidx = 0
                for ig in range(2):
                    rows = 128 if ig == 0 else 64
                    nblk = rows // 32
                    for jc in range(NJC):
                        jsl = slice(jc * JC, (jc + 1) * JC)
                        pms = []
                        for li in range(NL):
                            pm = mps_pool.tile([rows, JC], f32)
                            for cb in range(nblk):
                                blk = ig * 4 + cb
                                for t in range(NT):
                                    lt = li * NT + t
                                    h = h_pool.tile([128, JC], bf)
                                    bcol = bias_sb[:, lt * NBLK + blk: lt * NBLK + blk + 1]
                                    e = pat[eidx % len(pat)]
                                    eidx += 1
                                    if e == "A":
                                        nc.scalar.activation(
                                            h[:], uh_sb[:, lt, jsl],
                                            mybir.ActivationFunctionType.Relu,
                                            bias=bcol, scale=1.0)
                                    elif e == "G":
                                        nc.gpsimd.tensor_scalar(
                                            h[:], uh_sb[:, lt, jsl], bcol, 0.0,
                                            mybir.AluOpType.add, mybir.AluOpType.max)
                                    else:
                                        nc.vector.tensor_scalar(
                                            h[:], uh_sb[:, lt, jsl], bcol, 0.0,
                                            mybir.AluOpType.add, mybir.AluOpType.max)
                                    nc.tensor.matmul(
                                        pm[32 * cb:32 * (cb + 1), :],
                                        lhsT=wblk_sb[:, lt * 32:(lt + 1) * 32],
                                        rhs=h[:],
                                        start=(t == 0), stop=(t == NT - 1),
                                        tile_position=(0, 32 * cb),
                                        skip_group_check=True,
                                    )
                            pms.append(pm)
                        conv_t = cv_pool.tile([rows, JC], f32, tag="conv")
                        tmp_t = cv_pool.tile([rows, JC], f32, tag="tmp")
                        nc.vector.tensor_mul(conv_t[:], pms[0][:], ar[ig][:, jsl])
                        nc.vector.tensor_mul(tmp_t[:], pms[1][:], a2[ig][:, jsl])
                        nc.vector.tensor_add(conv_t[:], conv_t[:], tmp_t[:])
                        nc.sync.dma_start(
                            out=conv_d[ig * 128: ig * 128 + rows, jsl], in_=conv_t[:])

    nc.compile()
    return nc


def _host_prep(A, P, W1, b1, W2, b2):
    """Build per-core input maps. All heavy math stays on device; host only
    computes U = P @ W1 (O(N*32*32)) and reshuffles small arrays."""
    p = np.arange(128)
    it, ki = p // 4, p % 4

    in_maps = []
    # core-independent pieces
    uh_all = np.empty((NL * NT, 128, N), BF16)
    wblk = np.zeros((128, NL * NT * 32), np.float32)
    U = {}
    for li in range(NL):
        l = li + 1
        Ul = (P @ W1[l]).astype(np.float32)  # [N, 32]
        U[li] = Ul
        for t in range(NT):
            lt = li * NT + t
            uh_all[lt] = (-(Ul[:, 4 * t + ki].T)).astype(BF16)
            w = np.zeros((128, 32), np.float32)
            w[p, it] = W2[l, 4 * t + ki, 0]
            wblk[:, lt * 32:(lt + 1) * 32] = w
    wblk = wblk.astype(BF16)

    for c in range(NC):
        r0 = c * R
        bias = np.empty((128, NL * NT * NBLK), np.float32)
        for li in range(NL):
            l = li + 1
            Ul = U[li]
            for t in range(NT):
                lt = li * NT + t
                for blk in range(NBLK):
                    bias[:, lt * NBLK + blk] = (
                        Ul[r0 + 32 * blk + it, 4 * t + ki] + b1[l, 4 * t + ki])
        in_maps.append({
            "a_full": A,
            "a_colT": np.ascontiguousarray(A[:, r0:r0 + R]),
            "a_rows": np.ascontiguousarray(A[r0:r0 + R, :]),
            "uh": uh_all,
            "bias": bias,
            "wblk": wblk,
        })
    return in_maps


def kernel(A_norm, P, W1, b1, W2, b2):
    A = np.ascontiguousarray(np.asarray(A_norm, dtype=np.float32))
    P_ = np.ascontiguousarray(np.asarray(P, dtype=np.float32))
    W1_ = np.asarray(W1, dtype=np.float32)
    b1_ = np.asarray(b1, dtype=np.float32)
    W2_ = np.asarray(W2, dtype=np.float32)
    b2_ = np.asarray(b2, dtype=np.float32)

    assert A.shape == (N, N) and P_.shape == (N, KP)
    assert W1_.shape == (L, KP, FTS) and W2_.shape == (L, FTS, 1)

    if "prog" not in _PROG_CACHE:
        _PROG_CACHE["prog"] = _build_program()
    nc = _PROG_CACHE["prog"]

    in_maps = _host_prep(A, P_, W1_, b1_, W2_, b2_)
    res = run_bass_kernel_spmd(nc, in_maps, list(range(NC)))

    out = np.empty((N, N), np.float32)
    for c in range(NC):
        out[c * R:(c + 1) * R, :] = res.results[c]["conv_out"]

    # b2 for l=1,2 (additive constant per l, gated by A^l) — almost always 0.
    if b2_[1, 0] != 0.0 or b2_[2, 0] != 0.0:
        A2 = A @ A
        out += b2_[1, 0] * A + b2_[2, 0] * A2

    # l=0 term: A^0 = I, MLP_0(0) = relu(b1[0]) @ W2[0] + b2[0] on the diagonal
    c0 = float(np.maximum(b1_[0], 0.0) @ W2_[0][:, 0] + b2_[0, 0])
    if c0 != 0.0:
        idx = np.arange(N)
        out[idx, idx] += c0
    return out
